# revision 1
# baseline (speedup 1.0000x reference)
"""Trainium2 Bass kernel for nn_ConvolutionalCapsules.

Sharding: core c (of 8) owns output-capsule nout=c. Each core runs the p4 group
conv restricted to its 64 output channels (16 dout x 4 rot) over all 32 images
(B*Nin), then LayerNorm + degree-score routing + squash for its nout.

Conv: 3x3 conv as shifted matmuls from a zero-padded SBUF image (34x34 rows).
Partitions 0-63 hold the padded image (copy A), partitions 64-127 hold the same
image shifted by one padded row (copy B), so one K=128 matmul covers two filter
taps: (0,kx) on A plus (1,kx) on B at base offset kx. Row-2 taps run as K=64
matmuls on copy A. 6 matmuls accumulate one PSUM tile of 512 positions.

Routing runs in a transposed layout (positions on partitions, (i,d,g) on the
free axis) produced by PE transpose-mode, so every reduction (d, i, g) is a
free-axis tensor_reduce.
"""

import numpy as np
from contextlib import ExitStack

import concourse.bass as bass
import concourse.tile as tile
from concourse import mybir
from concourse.bass_utils import run_bass_kernel_spmd

F32 = mybir.dt.float32
F32R = mybir.dt.float32r
AF = mybir.ActivationFunctionType
OP = mybir.AluOpType
AX = mybir.AxisListType

MM_DT = F32R  # float32r: full-rate PE at ~tf32 precision; set F32 for exact

_ENGINES = {
    mybir.EngineType.PE,
    mybir.EngineType.Activation,
    mybir.EngineType.Pool,
    mybir.EngineType.DVE,
    mybir.EngineType.SP,
}


def _split_sync_waits(nc):
    """This walrus build accepts a single embedded sync-wait per instruction;
    hoist extras onto preceding NoOps on the same engine (ge-imm waits commute)."""
    for f in nc.m.functions:
        for bb in f.blocks:
            newl = []
            changed = False
            for inst in list(bb.instructions):
                si = inst.sync_info
                waits = list(si.on_wait) if si and si.on_wait else []
                if len(waits) > 1 and inst.engine in _ENGINES:
                    changed = True
                    for k, w in enumerate(waits[:-1]):
                        newl.append(
                            mybir.InstNoOp(
                                name=f"{inst.name}-ws{k}",
                                ins=[],
                                outs=[],
                                engine=inst.engine,
                                sync_info=mybir.SyncInfo(on_wait=[w], on_update=[]),
                            )
                        )
                    si.on_wait = waits[-1:]
                    inst.sync_info = si
                newl.append(inst)
            if changed:
                bb.instructions = newl


def build_program(apply_bias=False, apply_gb=False):
    nc = bass.Bass(trn_type="TRN2")
    caps = nc.dram_tensor("caps", [4, 8, 16, 4, 32, 32], MM_DT, kind="ExternalInput")
    w = nc.dram_tensor("w", [128, 384], MM_DT, kind="ExternalInput")
    ident = nc.dram_tensor("ident", [128, 128], F32, kind="ExternalInput")
    zer = nc.dram_tensor("zer", [1, 1164], MM_DT, kind="ExternalInput")
    if apply_bias:
        cb = nc.dram_tensor("cb", [64, 1], F32, kind="ExternalInput")
    if apply_gb:
        gam = nc.dram_tensor("gam", [1, 16], F32, kind="ExternalInput")
        bet = nc.dram_tensor("bet", [1, 16], F32, kind="ExternalInput")
    out = nc.dram_tensor("out", [4, 16, 4, 32, 32], F32, kind="ExternalOutput")

    caps_r = caps.ap().rearrange("b n d g h w -> (b n) (d g) h w")  # [32,64,32,32]
    out_r = out.ap().rearrange("b d g h w -> b (h w) d g")  # [4,1024,16,4]

    XW = 1164  # padded 34x34 image (1156) + slack so 16-row AP views stay in-bounds

    with tile.TileContext(nc) as tc:
        with ExitStack() as ctx:
            singles = ctx.enter_context(tc.tile_pool(name="singles", bufs=1))
            ps_conv = ctx.enter_context(tc.tile_pool(name="ps_conv", bufs=4, space="PSUM"))
            ps_tr = ctx.enter_context(tc.tile_pool(name="ps_tr", bufs=3, space="PSUM"))
            tpool = ctx.enter_context(tc.tile_pool(name="tpool", bufs=3))
            rbig = ctx.enter_context(tc.tile_pool(name="rbig", bufs=3))
            sm = ctx.enter_context(tc.tile_pool(name="sm", bufs=3))
            vout = ctx.enter_context(tc.tile_pool(name="vout", bufs=2))

            w_sb = singles.tile([128, 384], MM_DT, tag="w")
            nc.sync.dma_start(out=w_sb[:], in_=w.ap())
            id_sb = singles.tile([128, 128], F32, tag="ident")
            nc.sync.dma_start(out=id_sb[:], in_=ident.ap())
            if apply_bias:
                cb_sb = singles.tile([64, 1], F32, tag="cb")
                nc.sync.dma_start(out=cb_sb[:], in_=cb.ap())
            if apply_gb:
                gam_sb = singles.tile([128, 16], F32, tag="gam")
                nc.sync.dma_start(out=gam_sb[:], in_=gam.ap().partition_broadcast(128))
                bet_sb = singles.tile([128, 16], F32, tag="bet")
                nc.sync.dma_start(out=bet_sb[:], in_=bet.ap().partition_broadcast(128))

            eps5 = singles.tile([128, 1], F32, tag="eps5")
            nc.vector.memset(eps5[:], 1e-5)
            eps16 = singles.tile([128, 1], F32, tag="eps16")
            nc.vector.memset(eps16[:], 1e-16)

            xpads = []
            for ix in range(3):
                xp = singles.tile([128, XW], MM_DT, tag=f"xpad{ix}", name=f"xpad{ix}")
                nc.sync.dma_start(out=xp[:], in_=zer.ap().partition_broadcast(128))
                xpads.append(xp)

            u_sb = [
                [singles.tile([128, 1024], F32, tag=f"u{b}_{p}", name=f"u{b}_{p}") for p in range(4)]
                for b in range(4)
            ]

            def hview(ap_flat, o, rows):
                """[P, rows, 32] window at flat offset o with padded row stride 34."""
                return ap_flat[:, o: o + rows * 34].rearrange(
                    "c (h w) -> c h w", w=34
                )[:, :, 0:32]

            for b in range(4):
                # ---- conv for the 8 images of this batch ----
                for n in range(8):
                    img = b * 8 + n
                    xp = xpads[img % 3]
                    src = caps_r[img]  # [64,32,32]
                    dstA = hview(xp[0:64], 35, 32)
                    dstB = hview(xp[64:128], 1, 32)
                    nc.sync.dma_start(out=dstA, in_=src)
                    nc.sync.dma_start(out=dstB, in_=src)
                    pair, half = n // 2, n % 2
                    for chh in range(2):
                        ps = ps_conv.tile([64, 512], F32, tag="ps")
                        base = chh * 16 * 34
                        for kx in range(3):
                            rhs = hview(xp, base + kx, 16)
                            nc.tensor.matmul(
                                ps[:],
                                lhsT=w_sb[:, kx * 64:(kx + 1) * 64],
                                rhs=rhs,
                                start=(kx == 0),
                                stop=False,
                            )
                        for kx in range(3):
                            rhs = hview(xp[0:64], base + 68 + kx, 16)
                            nc.tensor.matmul(
                                ps[:],
                                lhsT=w_sb[0:64, (3 + kx) * 64:(4 + kx) * 64],
                                rhs=rhs,
                                start=False,
                                stop=(kx == 2),
                            )
                        dst = u_sb[b][pair][half * 64:(half + 1) * 64, chh * 512:(chh + 1) * 512]
                        if apply_bias:
                            nc.scalar.activation(dst, ps[:], AF.Identity, bias=cb_sb[:], scale=1.0)
                        else:
                            nc.scalar.activation(dst, ps[:], AF.Copy)

                # ---- transpose + routing, two steps of 4 position-blocks ----
                for sh in range(2):
                    T = tpool.tile([128, 2048], F32, tag="T")
                    for bq in range(4):
                        blk = sh * 4 + bq
                        pst = ps_tr.tile([128, 512], F32, tag="pst")
                        for p in range(4):
                            nc.tensor.transpose(
                                out=pst[:, p * 128:(p + 1) * 128],
                                in_=u_sb[b][p][:, blk * 128:(blk + 1) * 128],
                                identity=id_sb[:],
                            )
                        nc.scalar.activation(T[:, bq * 512:(bq + 1) * 512], pst[:], AF.Copy)

                    # views: col = k*512 + i*64 + d*4 + g
                    T5 = T.rearrange("p (k i d g) -> p k i d g", k=4, i=8, d=16)

                    mu = sm.tile([128, 128], F32, tag="mu")
                    mu4 = mu.rearrange("p (k i g) -> p k i g", k=4, i=8)
                    nc.vector.reduce_sum(mu4, T5.transpose((0, 1, 2, 4, 3)), AX.X)

                    sq = rbig.tile([128, 2048], F32, tag="scratch")
                    nc.scalar.activation(sq[:], T[:], AF.Square)
                    sq5 = sq.rearrange("p (k i d g) -> p k i d g", k=4, i=8, d=16)
                    msq = sm.tile([128, 128], F32, tag="msq")
                    msq4 = msq.rearrange("p (k i g) -> p k i g", k=4, i=8)
                    nc.vector.reduce_sum(msq4, sq5.transpose((0, 1, 2, 4, 3)), AX.X)

                    m1 = sm.tile([128, 128], F32, tag="m1")
                    nc.vector.tensor_scalar_mul(out=m1[:], in0=mu[:], scalar1=1.0 / 16.0)
                    var = sm.tile([128, 128], F32, tag="var")
                    nc.vector.tensor_tensor(out=var[:], in0=m1[:], in1=m1[:], op=OP.mult)
                    nc.vector.scalar_tensor_tensor(
                        out=var[:], in0=msq[:], scalar=1.0 / 16.0, in1=var[:],
                        op0=OP.mult, op1=OP.subtract,
                    )
                    rstd = sm.tile([128, 128], F32, tag="rstd")
                    nc.scalar.activation(rstd[:], var[:], AF.Sqrt, bias=eps5[:])
                    nc.vector.reciprocal(rstd[:], rstd[:])
                    n2 = sm.tile([128, 128], F32, tag="n2")
                    nc.vector.tensor_tensor(out=n2[:], in0=m1[:], in1=rstd[:], op=OP.mult)

                    def bc_kig(t):  # [128,128] (k,i,g) -> [p,k,i,d,g]
                        return (
                            t.rearrange("p (k i g) -> p k i g", k=4, i=8)
                            .unsqueeze(3)
                            .broadcast_to((128, 4, 8, 16, 4))
                        )

                    up = rbig.tile([128, 2048], F32, tag="up")
                    up5 = up.rearrange("p (k i d g) -> p k i d g", k=4, i=8, d=16)
                    nc.vector.tensor_tensor(out=up5, in0=T5, in1=bc_kig(rstd), op=OP.mult)
                    nc.vector.tensor_tensor(out=up5, in0=up5, in1=bc_kig(n2), op=OP.subtract)
                    if apply_gb:
                        gb = gam_sb[:].unsqueeze(1).unsqueeze(2).unsqueeze(4).broadcast_to((128, 4, 8, 16, 4))
                        bb_ = bet_sb[:].unsqueeze(1).unsqueeze(2).unsqueeze(4).broadcast_to((128, 4, 8, 16, 4))
                        nc.vector.tensor_tensor(out=up5, in0=up5, in1=gb, op=OP.mult)
                        nc.vector.tensor_tensor(out=up5, in0=up5, in1=bb_, op=OP.add)

                    S = sm.tile([128, 256], F32, tag="S")
                    S4 = S.rearrange("p (k d g) -> p k d g", k=4, d=16)
                    nc.vector.reduce_sum(S4, up5.transpose((0, 1, 3, 4, 2)), AX.X)

                    P = rbig.tile([128, 2048], F32, tag="scratch")
                    P5 = P.rearrange("p (k i d g) -> p k i d g", k=4, i=8, d=16)
                    S_bc = S4.unsqueeze(2).broadcast_to((128, 4, 8, 16, 4))
                    nc.vector.tensor_tensor(out=P5, in0=up5, in1=S_bc, op=OP.mult)
                    dot = sm.tile([128, 128], F32, tag="dot")
                    dot4 = dot.rearrange("p (k i g) -> p k i g", k=4, i=8)
                    nc.vector.reduce_sum(dot4, P5.transpose((0, 1, 2, 4, 3)), AX.X)

                    ns = sm.tile([128, 128], F32, tag="ns")
                    nc.vector.tensor_tensor(out=ns[:], in0=rstd[:], in1=rstd[:], op=OP.mult)
                    nc.vector.scalar_tensor_tensor(
                        out=ns[:], in0=var[:], scalar=16.0, in1=ns[:],
                        op0=OP.mult, op1=OP.mult,
                    )
                    nc.vector.reciprocal(ns[:], ns[:])
                    rr = sm.tile([128, 128], F32, tag="rr")
                    nc.vector.tensor_tensor(out=rr[:], in0=dot[:], in1=ns[:], op=OP.mult)

                    rr4 = rr.rearrange("p (k i g) -> p k i g", k=4, i=8)
                    mx = sm.tile([128, 16], F32, tag="mx")
                    mx3 = mx.rearrange("p (k g) -> p k g", k=4)
                    nc.vector.reduce_max(mx3, rr4.transpose((0, 1, 3, 2)), AX.X)
                    es = sm.tile([128, 128], F32, tag="es")
                    es4 = es.rearrange("p (k i g) -> p k i g", k=4, i=8)
                    mx_bc = mx3.unsqueeze(2).broadcast_to((128, 4, 8, 4))
                    nc.vector.tensor_tensor(out=es4, in0=rr4, in1=mx_bc, op=OP.subtract)
                    nc.scalar.activation(es[:], es[:], AF.Exp)
                    Z = sm.tile([128, 16], F32, tag="Z")
                    Z3 = Z.rearrange("p (k g) -> p k g", k=4)
                    nc.vector.reduce_sum(Z3, es4.transpose((0, 1, 3, 2)), AX.X)
                    nc.vector.reciprocal(Z[:], Z[:])
                    sc = sm.tile([128, 128], F32, tag="sc")
                    sc4 = sc.rearrange("p (k i g) -> p k i g", k=4, i=8)
                    Z_bc = Z3.unsqueeze(2).broadcast_to((128, 4, 8, 4))
                    nc.vector.tensor_tensor(out=sc4, in0=es4, in1=Z_bc, op=OP.mult)

                    nc.vector.tensor_tensor(out=P5, in0=up5, in1=bc_kig(sc), op=OP.mult)
                    s_t = sm.tile([128, 256], F32, tag="s")
                    s4 = s_t.rearrange("p (k d g) -> p k d g", k=4, d=16)
                    nc.vector.reduce_sum(s4, P5.transpose((0, 1, 3, 4, 2)), AX.X)

                    ssq = sm.tile([128, 256], F32, tag="ssq")
                    nc.scalar.activation(ssq[:], s_t[:], AF.Square)
                    nsq = sm.tile([128, 64], F32, tag="nsq")
                    nsq3 = nsq.rearrange("p (k d) -> p k d", k=4)
                    nc.vector.reduce_sum(nsq3, ssq.rearrange("p (k d g) -> p k d g", k=4, d=16), AX.X)
                    sq1 = sm.tile([128, 64], F32, tag="sq1")
                    nc.scalar.activation(sq1[:], nsq[:], AF.Sqrt, bias=eps16[:])
                    nc.vector.scalar_tensor_tensor(
                        out=sq1[:], in0=nsq[:], scalar=1.0, in1=sq1[:],
                        op0=OP.add, op1=OP.mult,
                    )
                    nc.vector.reciprocal(sq1[:], sq1[:])
                    f = sm.tile([128, 64], F32, tag="f")
                    nc.vector.tensor_tensor(out=f[:], in0=nsq[:], in1=sq1[:], op=OP.mult)

                    v = vout.tile([128, 256], F32, tag="v")
                    v4 = v.rearrange("p (k d g) -> p k d g", k=4, d=16)
                    f_bc = f.rearrange("p (k d) -> p k d", k=4).unsqueeze(3).broadcast_to((128, 4, 16, 4))
                    nc.vector.tensor_tensor(out=v4, in0=s4, in1=f_bc, op=OP.mult)

                    dstv = out_r[b].rearrange("(kk p) d g -> p kk d g", p=128)
                    for kk in range(4):
                        nc.sync.dma_start(
                            out=dstv[:, sh * 4 + kk, :, :], in_=v4[:, kk, :, :]
                        )

    _split_sync_waits(nc)
    return nc


def _pack_weights(conv_w):
    w = np.asarray(conv_w, np.float32)
    wt = np.stack(
        [np.roll(np.rot90(w, k=r, axes=(3, 4)), r, axis=2) for r in range(4)], axis=1
    )
    W512 = np.ascontiguousarray(wt.reshape(512, 64, 3, 3), dtype=np.float32)
    packs = []
    for c in range(8):
        Wc = W512[64 * c: 64 * c + 64]
        w_pack = np.zeros((128, 6, 64), np.float32)
        for kx in range(3):
            w_pack[0:64, kx] = Wc[:, :, 0, kx].T
            w_pack[64:128, kx] = Wc[:, :, 1, kx].T
            w_pack[0:64, 3 + kx] = Wc[:, :, 2, kx].T
        packs.append(np.ascontiguousarray(w_pack.reshape(128, 384)))
    return packs


_CACHE = {}


def kernel(capsules, conv_w, conv_b, ln_gamma, ln_beta):
    capsules = np.ascontiguousarray(np.asarray(capsules, np.float32))
    conv_b = np.asarray(conv_b, np.float32)
    ln_gamma = np.asarray(ln_gamma, np.float32)
    ln_beta = np.asarray(ln_beta, np.float32)
    apply_bias = bool(np.any(conv_b))
    apply_gb = bool(np.any(ln_gamma != 1.0) or np.any(ln_beta != 0.0))

    key = (apply_bias, apply_gb)
    if key not in _CACHE:
        _CACHE[key] = build_program(apply_bias=apply_bias, apply_gb=apply_gb)
    nc = _CACHE[key]

    packs = _pack_weights(conv_w)
    ident = np.eye(128, dtype=np.float32)
    in_maps = []
    for c in range(8):
        m = {"caps": capsules, "w": packs[c], "ident": ident,
             "zer": np.zeros((1, 1164), np.float32)}
        if apply_bias:
            b_loc = np.repeat(conv_b[c * 16:(c + 1) * 16], 4)  # partition = d*4+g
            m["cb"] = np.ascontiguousarray(b_loc.reshape(64, 1))
        if apply_gb:
            m["gam"] = np.ascontiguousarray(ln_gamma.reshape(1, 16))
            m["bet"] = np.ascontiguousarray(ln_beta.reshape(1, 16))
        in_maps.append(m)

    res = run_bass_kernel_spmd(nc, in_maps, core_ids=list(range(8)), trace=False)
    out = np.stack([res.results[c]["out"] for c in range(8)], axis=1)
    return out.astype(np.float32)



# revision 5
# speedup vs baseline: 2.0803x; 2.0803x over previous
"""Trainium2 Bass kernel for nn_ConvolutionalCapsules.

Sharding: core c (of 8) owns output-capsules {2*(c%4), 2*(c%4)+1} for batches
{2*(c//4), 2*(c//4)+1}. Each core runs the p4 group conv with 128 output
channels (2 nout x 16 dout x 4 rot) over its 16 images (2 batches x 8 input
capsules), then LayerNorm + degree-score routing + squash per (batch, nout).

Conv: 3x3 conv as shifted matmuls from a zero-padded fp16 SBUF image (34x34
rows, dual-copy: partitions 0-63 = padded image, 64-127 = same shifted one row
so one K=128 matmul covers two filter taps). Weights are the stationary
operand ([K, 128 out-channels]), so each of the 6 matmuls per 512 positions
retires 2x the baseline's work. PE then transposes u ([128 chan, pos]) to the
routing layout ([128 pos, chan]) in fp16 (4x cheaper than f32 transposes).

Routing runs fully in fp16 on the DVE (tensor_tensor at the 2x perf mode,
tensor_scalar at 4x); segmented reductions over d/i/g are binary tree-adds of
strided views, which beat TensorReduce ~4x. LayerNorm is algebraically folded:
up = (T - mu)*rstd, rr_i = (up_i . S) * (var+eps)/(16 var), softmax over i
(shift-free: |rr| <= 8), s = sum_i score_i up_i, squash over g.

Host packs inputs (pad + dual-copy + fp16) and unpacks the [pos, (b,n,k,d,g)]
fp16 output, so every DMA moves contiguous >=512B lines.
"""

import numpy as np
from contextlib import ExitStack

import concourse.bass as bass
import concourse.tile as tile
from concourse import mybir
from concourse.bass_utils import run_bass_kernel_spmd

F16 = mybir.dt.float16
F32 = mybir.dt.float32
AF = mybir.ActivationFunctionType
OP = mybir.AluOpType

_ENGINES = {
    mybir.EngineType.PE,
    mybir.EngineType.Activation,
    mybir.EngineType.Pool,
    mybir.EngineType.DVE,
    mybir.EngineType.SP,
}


def _split_sync_waits(nc):
    """This walrus build accepts a single embedded sync-wait per instruction;
    hoist extras onto preceding NoOps on the same engine (ge-imm waits commute)."""
    for f in nc.m.functions:
        for bb in f.blocks:
            newl = []
            changed = False
            for inst in list(bb.instructions):
                si = inst.sync_info
                waits = list(si.on_wait) if si and si.on_wait else []
                if len(waits) > 1 and inst.engine in _ENGINES:
                    changed = True
                    for k, w in enumerate(waits[:-1]):
                        newl.append(
                            mybir.InstNoOp(
                                name=f"{inst.name}-ws{k}",
                                ins=[],
                                outs=[],
                                engine=inst.engine,
                                sync_info=mybir.SyncInfo(on_wait=[w], on_update=[]),
                            )
                        )
                    si.on_wait = waits[-1:]
                    inst.sync_info = si
                newl.append(inst)
            if changed:
                bb.instructions = newl


def build_program(apply_bias=False, apply_gb=False):
    nc = bass.Bass(trn_type="TRN2")
    capsd = nc.dram_tensor("capsd", [16, 128, 1164], F16, kind="ExternalInput")
    w = nc.dram_tensor("w", [128, 768], F16, kind="ExternalInput")
    ident = nc.dram_tensor("ident", [128, 128], F16, kind="ExternalInput")
    if apply_bias:
        cb = nc.dram_tensor("cb", [1, 128], F16, kind="ExternalInput")
    if apply_gb:
        gam = nc.dram_tensor("gam", [1, 16], F16, kind="ExternalInput")
        bet = nc.dram_tensor("bet", [1, 16], F16, kind="ExternalInput")
    outd = nc.dram_tensor("outd", [128, 2048], F16, kind="ExternalOutput")

    with tile.TileContext(nc) as tc:
        with nc.allow_low_precision(reason="fp16 routing; 2e-2 rel tolerance"), \
             ExitStack() as ctx:
            consts = ctx.enter_context(tc.tile_pool(name="consts", bufs=1))
            imgs = ctx.enter_context(tc.tile_pool(name="imgs", bufs=1))
            us = ctx.enter_context(tc.tile_pool(name="us", bufs=1))
            ps = ctx.enter_context(tc.tile_pool(name="ps", bufs=3, space="PSUM"))
            tps = ctx.enter_context(tc.tile_pool(name="tps", bufs=4, space="PSUM"))
            Tp = ctx.enter_context(tc.tile_pool(name="Tp", bufs=1))
            qp = ctx.enter_context(tc.tile_pool(name="qp", bufs=2))
            scr = ctx.enter_context(tc.tile_pool(name="scr", bufs=2))
            trees = ctx.enter_context(tc.tile_pool(name="trees", bufs=2))
            sm = ctx.enter_context(tc.tile_pool(name="sm", bufs=2))
            vp = ctx.enter_context(tc.tile_pool(name="vp", bufs=2))

            w_sb = consts.tile([128, 768], F16, tag="w")
            nc.sync.dma_start(out=w_sb[:], in_=w.ap())
            id_sb = consts.tile([128, 128], F16, tag="ident")
            nc.sync.dma_start(out=id_sb[:], in_=ident.ap())
            eps5 = consts.tile([128, 1], F32, tag="eps5")
            nc.vector.memset(eps5[:], 1e-5)
            eps16 = consts.tile([128, 1], F32, tag="eps16")
            nc.vector.memset(eps16[:], 1e-16)
            if apply_bias:
                cb_sb = consts.tile([1, 128], F16, tag="cb")
                nc.sync.dma_start(out=cb_sb[:], in_=cb.ap())
                ones512 = consts.tile([1, 512], F16, tag="ones512")
                nc.vector.memset(ones512[:], 1.0)
            if apply_gb:
                gam_sb = consts.tile([128, 16], F16, tag="gam")
                nc.sync.dma_start(out=gam_sb[:], in_=gam.ap().partition_broadcast(128))
                bet_sb = consts.tile([128, 16], F16, tag="bet")
                nc.sync.dma_start(out=bet_sb[:], in_=bet.ap().partition_broadcast(128))

            def hview(ap_flat, o, rows):
                """[P, rows, 32] window at flat offset o, padded row stride 34."""
                return ap_flat[:, o: o + rows * 34].rearrange(
                    "c (h w) -> c h w", w=34
                )[:, :, 0:32]

            u_tiles = {}
            T_tiles = {}

            def conv(bl):
                for i in range(8):
                    m = bl * 8 + i
                    xi = imgs.tile([128, 1164], F16, tag=f"x{i}", name=f"x{i}")
                    nc.sync.dma_start(out=xi[:], in_=capsd.ap()[m])
                    u = us.tile([128, 1024], F16, tag=f"u{i}", name=f"u{i}")
                    u_tiles[m] = u
                    for chh in range(2):
                        p = ps.tile([128, 512], F32, tag="ps")
                        base = chh * 16 * 34
                        for kx in range(3):
                            nc.tensor.matmul(
                                p[:],
                                lhsT=w_sb[:, kx * 128:(kx + 1) * 128],
                                rhs=hview(xi, base + kx, 16),
                                start=(kx == 0), stop=False)
                        for kx in range(3):
                            last = (kx == 2) and not apply_bias
                            nc.tensor.matmul(
                                p[:],
                                lhsT=w_sb[0:64, (3 + kx) * 128:(4 + kx) * 128],
                                rhs=hview(xi[0:64], base + 68 + kx, 16),
                                start=False, stop=last)
                        if apply_bias:
                            nc.tensor.matmul(
                                p[:], lhsT=cb_sb[:], rhs=ones512[:],
                                start=False, stop=True)
                        nc.scalar.activation(
                            u[:, chh * 512:(chh + 1) * 512], p[:], AF.Copy)

            def trans(bl):
                T = Tp.tile([128, 8192], F16, tag=f"T{bl}", name=f"T{bl}")
                T_tiles[bl] = T
                for k in range(8):
                    for ih in range(2):
                        tp_ = tps.tile([128, 512], F16, tag="tps")
                        for j in range(4):
                            m = bl * 8 + ih * 4 + j
                            nc.tensor.transpose(
                                out=tp_[:, j * 128:(j + 1) * 128],
                                in_=u_tiles[m][:, k * 128:(k + 1) * 128],
                                identity=id_sb[:])
                        nc.scalar.activation(
                            T[:, k * 1024 + ih * 512: k * 1024 + (ih + 1) * 512],
                            tp_[:], AF.Copy)

            def v5(t, nd):
                return t.rearrange("p (k i d g) -> p k i d g", k=8, i=8, d=nd)

            def dtree(src5, out_kig, nd0=16):
                """Sum over the d axis of [P,k,i,d,g] via binary tree-adds."""
                cur = src5
                nd = nd0
                while nd > 2:
                    nd //= 2
                    t = trees.tile([128, 8 * 8 * nd * 4], F16, tag=f"d{nd}",
                                   name=f"d{nd}")
                    t5 = v5(t, nd)
                    nc.vector.tensor_tensor(
                        out=t5, in0=cur[:, :, :, 0:nd, :],
                        in1=cur[:, :, :, nd:2 * nd, :], op=OP.add)
                    cur = t5
                o5 = out_kig.rearrange("p (k i g) -> p k i g", k=8, i=8).unsqueeze(3)
                nc.vector.tensor_tensor(
                    out=o5, in0=cur[:, :, :, 0:1, :], in1=cur[:, :, :, 1:2, :],
                    op=OP.add)

            def itree(src5, out_kdg):
                """Sum over the i axis of [P,k,i,d,g] via binary tree-adds."""
                cur = src5
                ni = 8
                while ni > 2:
                    ni //= 2
                    t = trees.tile([128, 8 * ni * 16 * 4], F16, tag=f"i{ni}",
                                   name=f"i{ni}")
                    t5 = t.rearrange("p (k i d g) -> p k i d g", k=8, i=ni, d=16)
                    nc.vector.tensor_tensor(
                        out=t5, in0=cur[:, :, 0:ni, :, :],
                        in1=cur[:, :, ni:2 * ni, :, :], op=OP.add)
                    cur = t5
                o5 = out_kdg.rearrange("p (k d g) -> p k d g", k=8, d=16).unsqueeze(2)
                nc.vector.tensor_tensor(
                    out=o5, in0=cur[:, :, 0:1, :, :], in1=cur[:, :, 1:2, :, :],
                    op=OP.add)

            def bc_kig(t):  # [128,256] (k,i,g) -> [p,k,i,d,g]
                return (t.rearrange("p (k i g) -> p k i g", k=8, i=8)
                        .unsqueeze(3).broadcast_to((128, 8, 8, 16, 4)))

            def bc_kdg(t):  # [128,512] (k,d,g) -> [p,k,i,d,g]
                return (t.rearrange("p (k d g) -> p k d g", k=8, d=16)
                        .unsqueeze(2).broadcast_to((128, 8, 8, 16, 4)))

            def route(bl, n):
                T6 = T_tiles[bl].rearrange(
                    "p (k i n d g) -> p k i n d g", k=8, i=8, n=2, d=16)
                T5 = T6[:, :, :, n, :, :]
                # LayerNorm stats
                mu = sm.tile([128, 256], F16, tag="mu")
                dtree(T5, mu)
                sq = scr.tile([128, 4096], F16, tag="sq")
                nc.vector.tensor_tensor(out=v5(sq, 16), in0=T5, in1=T5, op=OP.mult)
                msq = sm.tile([128, 256], F16, tag="msq")
                dtree(v5(sq, 16), msq)
                m1 = sm.tile([128, 256], F16, tag="m1")
                nc.vector.tensor_scalar_mul(out=m1[:], in0=mu[:], scalar1=1.0 / 16.0)
                mm_ = sm.tile([128, 256], F16, tag="mm_")
                nc.vector.tensor_tensor(out=mm_[:], in0=m1[:], in1=m1[:], op=OP.mult)
                var = sm.tile([128, 256], F16, tag="var")
                nc.vector.scalar_tensor_tensor(
                    out=var[:], in0=msq[:], scalar=1.0 / 16.0, in1=mm_[:],
                    op0=OP.mult, op1=OP.subtract)
                rstd = sm.tile([128, 256], F16, tag="rstd")
                nc.scalar.activation(rstd[:], var[:], AF.Sqrt, bias=eps5[:])
                nc.vector.reciprocal(rstd[:], rstd[:])
                n2 = sm.tile([128, 256], F16, tag="n2")
                nc.vector.tensor_tensor(out=n2[:], in0=m1[:], in1=rstd[:], op=OP.mult)

                # up = (T - m1) * rstd  (optionally * gamma + beta)
                q = qp.tile([128, 4096], F16, tag="q")
                q5 = v5(q, 16)
                nc.vector.tensor_tensor(out=q5, in0=T5, in1=bc_kig(rstd), op=OP.mult)
                up = qp.tile([128, 4096], F16, tag="up")
                up5 = v5(up, 16)
                nc.vector.tensor_tensor(out=up5, in0=q5, in1=bc_kig(n2), op=OP.subtract)
                if apply_gb:
                    gb = (gam_sb[:].unsqueeze(1).unsqueeze(2).unsqueeze(4)
                          .broadcast_to((128, 8, 8, 16, 4)))
                    bb_ = (bet_sb[:].unsqueeze(1).unsqueeze(2).unsqueeze(4)
                           .broadcast_to((128, 8, 8, 16, 4)))
                    nc.vector.tensor_tensor(out=up5, in0=up5, in1=gb, op=OP.mult)
                    nc.vector.tensor_tensor(out=up5, in0=up5, in1=bb_, op=OP.add)

                # S = sum_i up ; dot_i = up_i . S
                S = sm.tile([128, 512], F16, tag="S")
                itree(up5, S)
                P = scr.tile([128, 4096], F16, tag="P", name="P")
                P5 = v5(P, 16)
                nc.vector.tensor_tensor(out=P5, in0=up5, in1=bc_kdg(S), op=OP.mult)
                dot = sm.tile([128, 256], F16, tag="dot")
                dtree(P5, dot)

                # rr_i = dot_i / max(||up_i||^2, 1e-8)
                rr = sm.tile([128, 256], F16, tag="rr")
                if not apply_gb:
                    # ||up||^2 = 16 var/(var+eps) exactly
                    v16 = sm.tile([128, 256], F16, tag="v16")
                    nc.vector.tensor_scalar_mul(out=v16[:], in0=var[:], scalar1=16.0)
                    nc.vector.reciprocal(v16[:], v16[:])
                    va = sm.tile([128, 256], F16, tag="va")
                    nc.vector.tensor_scalar_add(out=va[:], in0=var[:], scalar1=1e-5)
                    nc.vector.tensor_tensor(out=rr[:], in0=dot[:], in1=va[:], op=OP.mult)
                    nc.vector.tensor_tensor(out=rr[:], in0=rr[:], in1=v16[:], op=OP.mult)
                else:
                    usq = scr.tile([128, 4096], F16, tag="sq")
                    nc.vector.tensor_tensor(out=usq[:], in0=up[:], in1=up[:], op=OP.mult)
                    nq = sm.tile([128, 256], F16, tag="nq")
                    dtree(v5(usq, 16), nq)
                    nc.vector.tensor_scalar_max(out=nq[:], in0=nq[:], scalar1=1e-8)
                    nc.vector.reciprocal(nq[:], nq[:])
                    nc.vector.tensor_tensor(out=rr[:], in0=dot[:], in1=nq[:], op=OP.mult)

                # softmax over i (shift-free: |rr| <= 8)
                es = sm.tile([128, 256], F16, tag="es")
                nc.scalar.activation(es[:], rr[:], AF.Exp)
                es4 = es.rearrange("p (k i g) -> p k i g", k=8, i=8)
                zt1 = trees.tile([128, 128], F16, tag="z4")
                z14 = zt1.rearrange("p (k i g) -> p k i g", k=8, i=4)
                nc.vector.tensor_tensor(out=z14, in0=es4[:, :, 0:4, :],
                                        in1=es4[:, :, 4:8, :], op=OP.add)
                zt2 = trees.tile([128, 64], F16, tag="z2")
                z24 = zt2.rearrange("p (k i g) -> p k i g", k=8, i=2)
                nc.vector.tensor_tensor(out=z24, in0=z14[:, :, 0:2, :],
                                        in1=z14[:, :, 2:4, :], op=OP.add)
                Z = sm.tile([128, 32], F16, tag="Z")
                Z4 = Z.rearrange("p (k g) -> p k g", k=8).unsqueeze(2)
                nc.vector.tensor_tensor(out=Z4, in0=z24[:, :, 0:1, :],
                                        in1=z24[:, :, 1:2, :], op=OP.add)
                nc.vector.reciprocal(Z[:], Z[:])
                sc = sm.tile([128, 256], F16, tag="sc")
                Zb = (Z.rearrange("p (k g) -> p k g", k=8).unsqueeze(2)
                      .broadcast_to((128, 8, 8, 4)))
                sc4 = sc.rearrange("p (k i g) -> p k i g", k=8, i=8)
                nc.vector.tensor_tensor(out=sc4, in0=es4, in1=Zb, op=OP.mult)

                # s = sum_i score_i up_i ; squash over g
                P2 = scr.tile([128, 4096], F16, tag="P", name="P2")
                P25 = v5(P2, 16)
                nc.vector.tensor_tensor(out=P25, in0=up5, in1=bc_kig(sc), op=OP.mult)
                s = sm.tile([128, 512], F16, tag="s")
                itree(P25, s)
                ssq = sm.tile([128, 512], F16, tag="ssq")
                nc.vector.tensor_tensor(out=ssq[:], in0=s[:], in1=s[:], op=OP.mult)
                s4 = ssq.rearrange("p (k d g) -> p k d g", k=8, d=16)
                gt = trees.tile([128, 256], F16, tag="g2")
                gt4 = gt.rearrange("p (k d g) -> p k d g", k=8, d=16)
                nc.vector.tensor_tensor(out=gt4, in0=s4[:, :, :, 0:2],
                                        in1=s4[:, :, :, 2:4], op=OP.add)
                nsq = sm.tile([128, 128], F16, tag="nsq")
                nsq4 = nsq.rearrange("p (k d) -> p k d", k=8).unsqueeze(3)
                nc.vector.tensor_tensor(out=nsq4, in0=gt4[:, :, :, 0:1],
                                        in1=gt4[:, :, :, 1:2], op=OP.add)
                sr = sm.tile([128, 128], F16, tag="sr")
                nc.scalar.activation(sr[:], nsq[:], AF.Sqrt, bias=eps16[:])
                d1 = sm.tile([128, 128], F16, tag="d1")
                nc.vector.scalar_tensor_tensor(
                    out=d1[:], in0=nsq[:], scalar=1.0, in1=sr[:],
                    op0=OP.add, op1=OP.mult)
                nc.vector.reciprocal(d1[:], d1[:])
                f = sm.tile([128, 128], F16, tag="f")
                nc.vector.tensor_tensor(out=f[:], in0=nsq[:], in1=d1[:], op=OP.mult)
                v = vp.tile([128, 512], F16, tag="v")
                v4 = v.rearrange("p (k d g) -> p k d g", k=8, d=16)
                fb = (f.rearrange("p (k d) -> p k d", k=8).unsqueeze(3)
                      .broadcast_to((128, 8, 16, 4)))
                s44 = s.rearrange("p (k d g) -> p k d g", k=8, d=16)
                nc.vector.tensor_tensor(out=v4, in0=s44, in1=fb, op=OP.mult)
                u_ = (bl * 2 + n) * 512
                nc.sync.dma_start(out=outd.ap()[:, u_:u_ + 512], in_=v[:])

            conv(0)
            trans(0)
            conv(1)
            route(0, 0)
            trans(1)
            route(0, 1)
            route(1, 0)
            route(1, 1)

    _split_sync_waits(nc)
    return nc


def _pack_weights(conv_w):
    w = np.asarray(conv_w, np.float32)
    wt = np.stack(
        [np.roll(np.rot90(w, k=r, axes=(3, 4)), r, axis=2) for r in range(4)], axis=1
    )
    W512 = np.ascontiguousarray(wt.reshape(512, 64, 3, 3), dtype=np.float32)
    packs = []
    for pi in range(4):
        Wc = W512[128 * pi: 128 * pi + 128]  # 2 nouts' channels (n,d,g)
        w_pack = np.zeros((128, 6, 128), np.float32)
        for kx in range(3):
            w_pack[0:64, kx] = Wc[:, :, 0, kx].T
            w_pack[64:128, kx] = Wc[:, :, 1, kx].T
            w_pack[0:64, 3 + kx] = Wc[:, :, 2, kx].T
        packs.append(np.ascontiguousarray(
            w_pack.reshape(128, 768), dtype=np.float16))
    return packs


def _pack_caps(capsules):
    """[4,8,16,4,32,32] f32 -> [32,128,1164] f16 (padded image + row-shifted
    copy per [128]-partition tile)."""
    x = np.asarray(capsules, np.float32).reshape(32, 64, 32, 32)
    pad = np.zeros((32, 64, 34, 34), np.float16)
    pad[:, :, 1:33, 1:33] = x.astype(np.float16)
    A = pad.reshape(32, 64, 1156)
    buf = np.zeros((32, 128, 1164), np.float16)
    buf[:, 0:64, 0:1156] = A
    buf[:, 64:128, 0:1122] = A[:, :, 34:1156]
    return buf


_CACHE = {}


def kernel(capsules, conv_w, conv_b, ln_gamma, ln_beta):
    conv_b = np.asarray(conv_b, np.float32)
    ln_gamma = np.asarray(ln_gamma, np.float32)
    ln_beta = np.asarray(ln_beta, np.float32)
    apply_bias = bool(np.any(conv_b))
    apply_gb = bool(np.any(ln_gamma != 1.0) or np.any(ln_beta != 0.0))

    key = (apply_bias, apply_gb)
    if key not in _CACHE:
        _CACHE[key] = build_program(apply_bias=apply_bias, apply_gb=apply_gb)
    nc = _CACHE[key]

    capsd = _pack_caps(capsules)
    packs = _pack_weights(conv_w)
    identity = np.eye(128, dtype=np.float16)
    in_maps = []
    for c in range(8):
        beta_ = c // 4   # batch-pair
        pi = c % 4       # nout-pair
        m = {"capsd": np.ascontiguousarray(capsd[16 * beta_: 16 * beta_ + 16]),
             "w": packs[pi], "ident": identity}
        if apply_bias:
            # channel order (n,d,g): n*64 + d*4 + g
            b_loc = np.repeat(conv_b[32 * pi: 32 * pi + 32], 4).astype(np.float16)
            m["cb"] = np.ascontiguousarray(b_loc.reshape(1, 128))
        if apply_gb:
            m["gam"] = np.ascontiguousarray(ln_gamma.reshape(1, 16), dtype=np.float16)
            m["bet"] = np.ascontiguousarray(ln_beta.reshape(1, 16), dtype=np.float16)
        in_maps.append(m)

    res = run_bass_kernel_spmd(nc, in_maps, core_ids=list(range(8)), trace=False)
    # per-core out: [128, 2048] f16 = (p, bl, n, k, d, g); position = k*128+p
    out = np.zeros((4, 8, 16, 4, 32, 32), np.float32)
    for c in range(8):
        beta_, pi = c // 4, c % 4
        r = np.asarray(res.results[c]["outd"], np.float32).reshape(128, 2, 2, 8, 16, 4)
        for bl in range(2):
            for n in range(2):
                out[2 * beta_ + bl, 2 * pi + n] = (
                    r[:, bl, n].transpose(2, 3, 1, 0).reshape(16, 4, 32, 32))
    return out


# revision 14
# speedup vs baseline: 2.4980x; 1.2008x over previous
"""Trainium2 Bass kernel for nn_ConvolutionalCapsules.

Sharding: core c (of 8) owns output-capsules {2*(c%4), 2*(c%4)+1} for batches
{2*(c//4), 2*(c//4)+1}. Each core runs the p4 group conv with 128 output
channels (2 nout x 16 dout x 4 rot) over its 16 images (2 batches x 8 input
capsules), then LayerNorm + degree-score routing + squash per (batch, nout).

Conv: 3x3 conv as shifted matmuls from a zero-padded fp16 SBUF image (34x34
rows, dual-copy: partitions 0-63 = padded image, 64-127 = same shifted one row
so one K=128 matmul covers two filter taps). Weights are the stationary
operand ([K, 128 out-channels]), so each of the 6 matmuls per 512 positions
retires 2x the baseline's work. PE then transposes u ([128 chan, pos]) to the
routing layout ([128 pos, chan]) in fp16 (4x cheaper than f32 transposes).

Routing runs fully in fp16 on the DVE (tensor_tensor at the 2x perf mode,
tensor_scalar at 4x); segmented reductions over d/i/g are binary tree-adds of
strided views, which beat TensorReduce ~4x. LayerNorm is algebraically folded:
up = (T - mu)*rstd, rr_i = (up_i . S) * (var+eps)/(16 var), softmax over i
(shift-free: |rr| <= 8), s = sum_i score_i up_i, squash over g.

Host packs inputs (pad + dual-copy + fp16) and unpacks the [pos, (b,n,k,d,g)]
fp16 output, so every DMA moves contiguous >=512B lines.
"""

import numpy as np
from contextlib import ExitStack

import concourse.bass as bass
import concourse.tile as tile
from concourse import mybir
from concourse.bass_utils import run_bass_kernel_spmd

F16 = mybir.dt.float16
F32 = mybir.dt.float32
AF = mybir.ActivationFunctionType
OP = mybir.AluOpType

_ENGINES = {
    mybir.EngineType.PE,
    mybir.EngineType.Activation,
    mybir.EngineType.Pool,
    mybir.EngineType.DVE,
    mybir.EngineType.SP,
}


def _split_sync_waits(nc):
    """This walrus build accepts a single embedded sync-wait per instruction;
    hoist extras onto preceding NoOps on the same engine (ge-imm waits commute)."""
    for f in nc.m.functions:
        for bb in f.blocks:
            newl = []
            changed = False
            for inst in list(bb.instructions):
                si = inst.sync_info
                waits = list(si.on_wait) if si and si.on_wait else []
                if len(waits) > 1 and inst.engine in _ENGINES:
                    changed = True
                    for k, w in enumerate(waits[:-1]):
                        newl.append(
                            mybir.InstNoOp(
                                name=f"{inst.name}-ws{k}",
                                ins=[],
                                outs=[],
                                engine=inst.engine,
                                sync_info=mybir.SyncInfo(on_wait=[w], on_update=[]),
                            )
                        )
                    si.on_wait = waits[-1:]
                    inst.sync_info = si
                newl.append(inst)
            if changed:
                bb.instructions = newl


def build_program(apply_bias=False, apply_gb=False):
    nc = bass.Bass(trn_type="TRN2")
    capsd = nc.dram_tensor("capsd", [16, 128, 1164], F16, kind="ExternalInput")
    w = nc.dram_tensor("w", [128, 768], F16, kind="ExternalInput")
    ident = nc.dram_tensor("ident", [128, 128], F16, kind="ExternalInput")
    mmu = nc.dram_tensor("mmu", [128, 16], F16, kind="ExternalInput")
    if apply_bias:
        cb = nc.dram_tensor("cb", [1, 128], F16, kind="ExternalInput")
    if apply_gb:
        gam = nc.dram_tensor("gam", [1, 16], F16, kind="ExternalInput")
        bet = nc.dram_tensor("bet", [1, 16], F16, kind="ExternalInput")
    outd = nc.dram_tensor("outd", [128, 2048], F16, kind="ExternalOutput")

    with tile.TileContext(nc) as tc:
        with nc.allow_low_precision(reason="fp16 routing; 2e-2 rel tolerance"), \
             ExitStack() as ctx:
            consts = ctx.enter_context(tc.tile_pool(name="consts", bufs=1))
            imgs = ctx.enter_context(tc.tile_pool(name="imgs", bufs=1))
            us = ctx.enter_context(tc.tile_pool(name="us", bufs=1))
            ps = ctx.enter_context(tc.tile_pool(name="ps", bufs=3, space="PSUM"))
            tps = ctx.enter_context(tc.tile_pool(name="tps", bufs=2, space="PSUM"))
            sps = ctx.enter_context(tc.tile_pool(name="sps", bufs=2, space="PSUM"))
            Tp = ctx.enter_context(tc.tile_pool(name="Tp", bufs=1))
            qp = ctx.enter_context(tc.tile_pool(name="qp", bufs=2))
            scr = ctx.enter_context(tc.tile_pool(name="scr", bufs=2))
            trees = ctx.enter_context(tc.tile_pool(name="trees", bufs=2))
            sm = ctx.enter_context(tc.tile_pool(name="sm", bufs=2))
            vp = ctx.enter_context(tc.tile_pool(name="vp", bufs=2))

            w_sb = consts.tile([128, 768], F16, tag="w")
            nc.sync.dma_start(out=w_sb[:], in_=w.ap())
            id_sb = consts.tile([128, 128], F16, tag="ident")
            nc.sync.dma_start(out=id_sb[:], in_=ident.ap())
            mmu_sb = consts.tile([128, 16], F16, tag="mmu")
            nc.sync.dma_start(out=mmu_sb[:], in_=mmu.ap())
            eps5 = consts.tile([128, 1], F32, tag="eps5")
            nc.vector.memset(eps5[:], 1e-5)
            eps16 = consts.tile([128, 1], F32, tag="eps16")
            nc.vector.memset(eps16[:], 1e-16)
            if apply_bias:
                cb_sb = consts.tile([1, 128], F16, tag="cb")
                nc.sync.dma_start(out=cb_sb[:], in_=cb.ap())
                ones512 = consts.tile([1, 512], F16, tag="ones512")
                nc.vector.memset(ones512[:], 1.0)
            if apply_gb:
                gam_sb = consts.tile([128, 16], F16, tag="gam")
                nc.sync.dma_start(out=gam_sb[:], in_=gam.ap().partition_broadcast(128))
                bet_sb = consts.tile([128, 16], F16, tag="bet")
                nc.sync.dma_start(out=bet_sb[:], in_=bet.ap().partition_broadcast(128))

            def hview(ap_flat, o, rows):
                """[P, rows, 32] window at flat offset o, padded row stride 34."""
                return ap_flat[:, o: o + rows * 34].rearrange(
                    "c (h w) -> c h w", w=34
                )[:, :, 0:32]

            u_tiles = {}
            usq_tiles = {}
            T_tiles = {}
            stat_tiles = {}

            def conv(bl):
                for i in range(8):
                    m = bl * 8 + i
                    xi = imgs.tile([128, 1164], F16, tag=f"x{i}", name=f"x{i}")
                    nc.sync.dma_start(out=xi[:], in_=capsd.ap()[m])
                    u = us.tile([128, 1024], F16, tag=f"u{i}", name=f"u{i}")
                    u_tiles[m] = u
                    for chh in range(2):
                        p = ps.tile([128, 512], F32, tag="ps")
                        base = chh * 16 * 34
                        for kx in range(3):
                            nc.tensor.matmul(
                                p[:],
                                lhsT=w_sb[:, kx * 128:(kx + 1) * 128],
                                rhs=hview(xi, base + kx, 16),
                                start=(kx == 0), stop=False)
                        for kx in range(3):
                            last = (kx == 2) and not apply_bias
                            nc.tensor.matmul(
                                p[:],
                                lhsT=w_sb[0:64, (3 + kx) * 128:(4 + kx) * 128],
                                rhs=hview(xi[0:64], base + 68 + kx, 16),
                                start=False, stop=last)
                        if apply_bias:
                            nc.tensor.matmul(
                                p[:], lhsT=cb_sb[:], rhs=ones512[:],
                                start=False, stop=True)
                        nc.scalar.activation(
                            u[:, chh * 512:(chh + 1) * 512], p[:], AF.Copy)
                    # u^2 on the (otherwise idle) GPSIMD engine, for the PE
                    # mean-square stat matmuls
                    usq = us.tile([128, 1024], F16, tag=f"usq{i}", name=f"usq{i}")
                    usq_tiles[m] = usq
                    nc.gpsimd.tensor_tensor(out=usq[:], in0=u[:], in1=u[:],
                                            op=OP.mult)

            def trans(bl):
                """Transpose u to routing layout; also reduce per-capsule LN
                stats (mean, mean-square over d) on the PE via Mmu matmuls."""
                T = Tp.tile([128, 8192], F16, tag=f"T{bl}", name=f"T{bl}")
                T_tiles[bl] = T
                stats = Tp.tile([128, 1024], F16, tag=f"st{bl}", name=f"st{bl}")
                stat_tiles[bl] = stats
                for k in range(8):
                    tp_ = tps.tile([128, 1024], F16, tag="tps")
                    for j in range(8):
                        m = bl * 8 + j
                        nc.tensor.transpose(
                            out=tp_[:, j * 128:(j + 1) * 128],
                            in_=u_tiles[m][:, k * 128:(k + 1) * 128],
                            identity=id_sb[:])
                    nc.scalar.activation(
                        T[:, k * 1024:(k + 1) * 1024], tp_[:], AF.Copy)
                    sp_ = sps.tile([128, 128], F32, tag="sps")
                    for j in range(8):
                        m = bl * 8 + j
                        nc.tensor.matmul(
                            sp_[:, j * 16: j * 16 + 8],
                            lhsT=u_tiles[m][:, k * 128:(k + 1) * 128],
                            rhs=mmu_sb[:, 0:8], start=True, stop=True)
                        nc.tensor.matmul(
                            sp_[:, j * 16 + 8: j * 16 + 16],
                            lhsT=usq_tiles[m][:, k * 128:(k + 1) * 128],
                            rhs=mmu_sb[:, 8:16], start=True, stop=True)
                    nc.scalar.activation(
                        stats[:, k * 128:(k + 1) * 128], sp_[:], AF.Copy)

            def v5(t, nd):
                return t.rearrange("p (k i d g) -> p k i d g", k=8, i=8, d=nd)

            def dtree(src5, out_kig, nd0=16):
                """Sum over the d axis of [P,k,i,d,g] via binary tree-adds."""
                cur = src5
                nd = nd0
                while nd > 2:
                    nd //= 2
                    t = trees.tile([128, 8 * 8 * nd * 4], F16, tag=f"d{nd}",
                                   name=f"d{nd}")
                    t5 = v5(t, nd)
                    nc.vector.tensor_tensor(
                        out=t5, in0=cur[:, :, :, 0:nd, :],
                        in1=cur[:, :, :, nd:2 * nd, :], op=OP.add)
                    cur = t5
                o5 = out_kig.rearrange("p (k i g) -> p k i g", k=8, i=8).unsqueeze(3)
                nc.vector.tensor_tensor(
                    out=o5, in0=cur[:, :, :, 0:1, :], in1=cur[:, :, :, 1:2, :],
                    op=OP.add)

            def itree(src5, out_kdg):
                """Sum over the i axis of [P,k,i,d,g] via binary tree-adds."""
                cur = src5
                ni = 8
                while ni > 2:
                    ni //= 2
                    t = trees.tile([128, 8 * ni * 16 * 4], F16, tag=f"i{ni}",
                                   name=f"i{ni}")
                    t5 = t.rearrange("p (k i d g) -> p k i d g", k=8, i=ni, d=16)
                    nc.vector.tensor_tensor(
                        out=t5, in0=cur[:, :, 0:ni, :, :],
                        in1=cur[:, :, ni:2 * ni, :, :], op=OP.add)
                    cur = t5
                o5 = out_kdg.rearrange("p (k d g) -> p k d g", k=8, d=16).unsqueeze(2)
                nc.vector.tensor_tensor(
                    out=o5, in0=cur[:, :, 0:1, :, :], in1=cur[:, :, 1:2, :, :],
                    op=OP.add)

            def bc_kig(t):  # [128,256] (k,i,g) -> [p,k,i,d,g]
                return (t.rearrange("p (k i g) -> p k i g", k=8, i=8)
                        .unsqueeze(3).broadcast_to((128, 8, 8, 16, 4)))

            def bc_kdg(t):  # [128,512] (k,d,g) -> [p,k,i,d,g]
                return (t.rearrange("p (k d g) -> p k d g", k=8, d=16)
                        .unsqueeze(2).broadcast_to((128, 8, 8, 16, 4)))

            def route(bl, n):
                T6 = T_tiles[bl].rearrange(
                    "p (k i n d g) -> p k i n d g", k=8, i=8, n=2, d=16)
                T5 = T6[:, :, :, n, :, :]
                # LayerNorm stats from the PE Mmu matmuls: m1 = mean over d,
                # e2 = mean of squares over d
                st6 = stat_tiles[bl].rearrange(
                    "p (k i t n g) -> p k i t n g", k=8, i=8, t=2, n=2)
                m1v = st6[:, :, :, 0, n, :]
                e2v = st6[:, :, :, 1, n, :]
                mm_ = sm.tile([128, 256], F16, tag="mm_")
                mm4 = mm_.rearrange("p (k i g) -> p k i g", k=8, i=8)
                nc.vector.tensor_tensor(out=mm4, in0=m1v, in1=m1v, op=OP.mult)
                var = sm.tile([128, 256], F16, tag="var")
                var4 = var.rearrange("p (k i g) -> p k i g", k=8, i=8)
                nc.vector.tensor_tensor(out=var4, in0=e2v, in1=mm4, op=OP.subtract)
                rstd = sm.tile([128, 256], F16, tag="rstd")
                nc.scalar.activation(rstd[:], var[:], AF.Sqrt, bias=eps5[:])
                nc.vector.reciprocal(rstd[:], rstd[:])
                rstd4 = rstd.rearrange("p (k i g) -> p k i g", k=8, i=8)
                n2 = sm.tile([128, 256], F16, tag="n2")
                n24 = n2.rearrange("p (k i g) -> p k i g", k=8, i=8)
                nc.vector.tensor_tensor(out=n24, in0=m1v, in1=rstd4, op=OP.mult)

                # up = (T - m1) * rstd  (optionally * gamma + beta)
                q = qp.tile([128, 4096], F16, tag="q")
                q5 = v5(q, 16)
                nc.vector.tensor_tensor(out=q5, in0=T5, in1=bc_kig(rstd), op=OP.mult)
                up = qp.tile([128, 4096], F16, tag="up")
                up5 = v5(up, 16)
                nc.vector.tensor_tensor(out=up5, in0=q5, in1=bc_kig(n2), op=OP.subtract)
                if apply_gb:
                    gb = (gam_sb[:].unsqueeze(1).unsqueeze(2).unsqueeze(4)
                          .broadcast_to((128, 8, 8, 16, 4)))
                    bb_ = (bet_sb[:].unsqueeze(1).unsqueeze(2).unsqueeze(4)
                           .broadcast_to((128, 8, 8, 16, 4)))
                    nc.vector.tensor_tensor(out=up5, in0=up5, in1=gb, op=OP.mult)
                    nc.vector.tensor_tensor(out=up5, in0=up5, in1=bb_, op=OP.add)

                # S = sum_i up ; dot_i = up_i . S
                S = sm.tile([128, 512], F16, tag="S")
                itree(up5, S)
                P = scr.tile([128, 4096], F16, tag="P", name="P")
                P5 = v5(P, 16)
                nc.vector.tensor_tensor(out=P5, in0=up5, in1=bc_kdg(S), op=OP.mult)
                dot = sm.tile([128, 256], F16, tag="dot")
                dtree(P5, dot)

                # rr_i = dot_i / max(||up_i||^2, 1e-8)
                rr = sm.tile([128, 256], F16, tag="rr")
                if not apply_gb:
                    # 1/||up||^2 = (var+eps)/(16 var) = 1/16 + (eps/16)/var
                    ns_ = sm.tile([128, 256], F16, tag="ns_")
                    nc.vector.reciprocal(ns_[:], var[:])
                    nc.vector.tensor_scalar(
                        out=ns_[:], in0=ns_[:], scalar1=1e-5 / 16.0,
                        scalar2=1.0 / 16.0, op0=OP.mult, op1=OP.add)
                    nc.vector.tensor_tensor(out=rr[:], in0=dot[:], in1=ns_[:], op=OP.mult)
                else:
                    usq = scr.tile([128, 4096], F16, tag="sq")
                    nc.vector.tensor_tensor(out=usq[:], in0=up[:], in1=up[:], op=OP.mult)
                    nq = sm.tile([128, 256], F16, tag="nq")
                    dtree(v5(usq, 16), nq)
                    nc.vector.tensor_scalar_max(out=nq[:], in0=nq[:], scalar1=1e-8)
                    nc.vector.reciprocal(nq[:], nq[:])
                    nc.vector.tensor_tensor(out=rr[:], in0=dot[:], in1=nq[:], op=OP.mult)

                # softmax over i (shift-free: |rr| <= 8)
                es = sm.tile([128, 256], F16, tag="es")
                nc.scalar.activation(es[:], rr[:], AF.Exp)
                es4 = es.rearrange("p (k i g) -> p k i g", k=8, i=8)
                zt1 = trees.tile([128, 128], F16, tag="z4")
                z14 = zt1.rearrange("p (k i g) -> p k i g", k=8, i=4)
                nc.vector.tensor_tensor(out=z14, in0=es4[:, :, 0:4, :],
                                        in1=es4[:, :, 4:8, :], op=OP.add)
                zt2 = trees.tile([128, 64], F16, tag="z2")
                z24 = zt2.rearrange("p (k i g) -> p k i g", k=8, i=2)
                nc.vector.tensor_tensor(out=z24, in0=z14[:, :, 0:2, :],
                                        in1=z14[:, :, 2:4, :], op=OP.add)
                Z = sm.tile([128, 32], F16, tag="Z")
                Z4 = Z.rearrange("p (k g) -> p k g", k=8).unsqueeze(2)
                nc.vector.tensor_tensor(out=Z4, in0=z24[:, :, 0:1, :],
                                        in1=z24[:, :, 1:2, :], op=OP.add)
                nc.vector.reciprocal(Z[:], Z[:])
                sc = sm.tile([128, 256], F16, tag="sc")
                Zb = (Z.rearrange("p (k g) -> p k g", k=8).unsqueeze(2)
                      .broadcast_to((128, 8, 8, 4)))
                sc4 = sc.rearrange("p (k i g) -> p k i g", k=8, i=8)
                nc.vector.tensor_tensor(out=sc4, in0=es4, in1=Zb, op=OP.mult)

                # s = sum_i score_i up_i ; squash over g
                P2 = scr.tile([128, 4096], F16, tag="P", name="P2")
                P25 = v5(P2, 16)
                nc.vector.tensor_tensor(out=P25, in0=up5, in1=bc_kig(sc), op=OP.mult)
                s = sm.tile([128, 512], F16, tag="s")
                itree(P25, s)
                ssq = sm.tile([128, 512], F16, tag="ssq")
                nc.vector.tensor_tensor(out=ssq[:], in0=s[:], in1=s[:], op=OP.mult)
                s4 = ssq.rearrange("p (k d g) -> p k d g", k=8, d=16)
                gt = trees.tile([128, 256], F16, tag="g2")
                gt4 = gt.rearrange("p (k d g) -> p k d g", k=8, d=16)
                nc.vector.tensor_tensor(out=gt4, in0=s4[:, :, :, 0:2],
                                        in1=s4[:, :, :, 2:4], op=OP.add)
                nsq = sm.tile([128, 128], F16, tag="nsq")
                nsq4 = nsq.rearrange("p (k d) -> p k d", k=8).unsqueeze(3)
                nc.vector.tensor_tensor(out=nsq4, in0=gt4[:, :, :, 0:1],
                                        in1=gt4[:, :, :, 1:2], op=OP.add)
                sr = sm.tile([128, 128], F16, tag="sr")
                nc.scalar.activation(sr[:], nsq[:], AF.Sqrt, bias=eps16[:])
                d1 = sm.tile([128, 128], F16, tag="d1")
                nc.vector.scalar_tensor_tensor(
                    out=d1[:], in0=nsq[:], scalar=1.0, in1=sr[:],
                    op0=OP.add, op1=OP.mult)
                nc.vector.reciprocal(d1[:], d1[:])
                f = sm.tile([128, 128], F16, tag="f")
                nc.vector.tensor_tensor(out=f[:], in0=nsq[:], in1=d1[:], op=OP.mult)
                v = vp.tile([128, 512], F16, tag="v")
                v4 = v.rearrange("p (k d g) -> p k d g", k=8, d=16)
                fb = (f.rearrange("p (k d) -> p k d", k=8).unsqueeze(3)
                      .broadcast_to((128, 8, 16, 4)))
                s44 = s.rearrange("p (k d g) -> p k d g", k=8, d=16)
                nc.vector.tensor_tensor(out=v4, in0=s44, in1=fb, op=OP.mult)
                u_ = (bl * 2 + n) * 512
                nc.sync.dma_start(out=outd.ap()[:, u_:u_ + 512], in_=v[:])

            conv(0)
            trans(0)
            route(0, 0)
            conv(1)
            route(0, 1)
            trans(1)
            route(1, 0)
            route(1, 1)

    _split_sync_waits(nc)
    return nc


def _pack_weights(conv_w):
    w = np.asarray(conv_w, np.float32)
    wt = np.stack(
        [np.roll(np.rot90(w, k=r, axes=(3, 4)), r, axis=2) for r in range(4)], axis=1
    )
    W512 = np.ascontiguousarray(wt.reshape(512, 64, 3, 3), dtype=np.float32)
    packs = []
    for pi in range(4):
        Wc = W512[128 * pi: 128 * pi + 128]  # 2 nouts' channels (n,d,g)
        w_pack = np.zeros((128, 6, 128), np.float32)
        for kx in range(3):
            w_pack[0:64, kx] = Wc[:, :, 0, kx].T
            w_pack[64:128, kx] = Wc[:, :, 1, kx].T
            w_pack[0:64, 3 + kx] = Wc[:, :, 2, kx].T
        packs.append(np.ascontiguousarray(
            w_pack.reshape(128, 768), dtype=np.float16))
    return packs


def _pack_caps(capsules):
    """[4,8,16,4,32,32] f32 -> [32,128,1164] f16 (padded image + row-shifted
    copy per [128]-partition tile)."""
    x = np.asarray(capsules, np.float32).reshape(32, 64, 32, 32)
    pad = np.zeros((32, 64, 34, 34), np.float16)
    pad[:, :, 1:33, 1:33] = x.astype(np.float16)
    A = pad.reshape(32, 64, 1156)
    buf = np.zeros((32, 128, 1164), np.float16)
    buf[:, 0:64, 0:1156] = A
    buf[:, 64:128, 0:1122] = A[:, :, 34:1156]
    return buf


_CACHE = {}


def kernel(capsules, conv_w, conv_b, ln_gamma, ln_beta):
    conv_b = np.asarray(conv_b, np.float32)
    ln_gamma = np.asarray(ln_gamma, np.float32)
    ln_beta = np.asarray(ln_beta, np.float32)
    apply_bias = bool(np.any(conv_b))
    apply_gb = bool(np.any(ln_gamma != 1.0) or np.any(ln_beta != 0.0))

    key = (apply_bias, apply_gb)
    if key not in _CACHE:
        _CACHE[key] = build_program(apply_bias=apply_bias, apply_gb=apply_gb)
    nc = _CACHE[key]

    capsd = _pack_caps(capsules)
    packs = _pack_weights(conv_w)
    identity = np.eye(128, dtype=np.float16)
    mmu = np.zeros((128, 16), np.float16)
    for ch in range(128):
        nn_, gg = ch // 64, ch % 4
        for t in range(2):
            mmu[ch, t * 8 + nn_ * 4 + gg] = 1.0 / 16.0
    in_maps = []
    for c in range(8):
        beta_ = c // 4   # batch-pair
        pi = c % 4       # nout-pair
        m = {"capsd": np.ascontiguousarray(capsd[16 * beta_: 16 * beta_ + 16]),
             "w": packs[pi], "ident": identity, "mmu": mmu}
        if apply_bias:
            # channel order (n,d,g): n*64 + d*4 + g
            b_loc = np.repeat(conv_b[32 * pi: 32 * pi + 32], 4).astype(np.float16)
            m["cb"] = np.ascontiguousarray(b_loc.reshape(1, 128))
        if apply_gb:
            m["gam"] = np.ascontiguousarray(ln_gamma.reshape(1, 16), dtype=np.float16)
            m["bet"] = np.ascontiguousarray(ln_beta.reshape(1, 16), dtype=np.float16)
        in_maps.append(m)

    res = run_bass_kernel_spmd(nc, in_maps, core_ids=list(range(8)), trace=False)
    # per-core out: [128, 2048] f16 = (p, bl, n, k, d, g); position = k*128+p
    out = np.zeros((4, 8, 16, 4, 32, 32), np.float32)
    for c in range(8):
        beta_, pi = c // 4, c % 4
        r = np.asarray(res.results[c]["outd"], np.float32).reshape(128, 2, 2, 8, 16, 4)
        for bl in range(2):
            for n in range(2):
                out[2 * beta_ + bl, 2 * pi + n] = (
                    r[:, bl, n].transpose(2, 3, 1, 0).reshape(16, 4, 32, 32))
    return out
